# revision 8
# baseline (speedup 1.0000x reference)
"""Trainium2 Bass kernel for nn_BaselineModel_27298812133937.

Model: two [32,512] token sequences -> shared embedding [50000,512] ->
3 stacked bi-GRU layers (H=256, Keras reset_after) -> last states,
plus a leaks MLP branch, then BN/FC/BN/sigmoid head -> [32].

Sharding: the two sequences share GRU weights, so they merge into a
batch of 64. Each of the 8 cores takes 8 merged examples (4 code + 4
comment of the same original examples), runs the full network for its
shard with no cross-core communication, and computes the head for its
4 original examples. Host concatenates the 8x[4] outputs.

On-core layout: feature dim on partitions, batch on the free dim. The
recurrent matmul keeps Wh (bf16) stationary and streams the state.
v2 scan: xz and the recurrent h-bias are accumulated into PSUM by
identity-stationary matmuls (PE is cheap; the gate chain is the
bottleneck), the per-direction gate chain is 2 Act + 3 DVE + 2 GpSimd
ops, and the hidden state lives in a [P, KH, BC, U] SBUF ring staged
to DRAM in half-block DMAs instead of per-step writes.
"""

import os
import sys

import numpy as np

for _p in ("/opt/trn_rl_repo",):
    if os.path.isdir(_p) and _p not in sys.path:
        sys.path.insert(0, _p)

import concourse.bass as bass
import concourse.tile as tile
from concourse import bacc, mybir
from concourse.masks import make_identity

import ml_dtypes

FP32 = mybir.dt.float32
BF16 = mybir.dt.bfloat16
I32 = mybir.dt.int32
AF = mybir.ActivationFunctionType
OP = mybir.AluOpType
NP_BF16 = ml_dtypes.bfloat16

V, E, H, NLAY = 50000, 512, 256, 3
EPS = 1e-3
P = 128
JX = E // P        # 4  x-feature tiles
JG = 3 * H // P    # 6  gate tiles
JH = H // P        # 2  hidden tiles
KH = H // P        # 2  Wh contraction tiles
BC = 8             # merged examples per core
BCH = 4            # head (original) examples per core
NCORES = 8
U = 16             # scan steps per For_i iteration == xp time-block


def build_nc(T=512, n_layers=NLAY, use_for_i=True, staggered=True, debug=False):
    assert T % P == 0 and T % U == 0
    TB = T // U
    NCH = BC * (T // P)

    nc = bacc.Bacc("TRN2", target_bir_lowering=False, debug=debug)

    def din(name, shape, dt):
        return nc.declare_dram_parameter(name, list(shape), dt, False)

    emb = din("emb", [V, E], BF16)
    idxw = din("idxw", [P, NCH], I32)
    wx = din("wx", [n_layers, 2, JX, JG, P, P], BF16)
    wh = din("wh", [n_layers, 2, KH, JG, P, P], BF16)
    pbias = din("pbias", [P, n_layers, 2, JG], FP32)
    b1hbc = din("b1hbc", [P, n_layers, 2, JH, BC], BF16)
    w1 = din("w1", [10, 2, P, P], BF16)
    b1p = din("b1p", [P, 2], FP32)
    wc = din("wc", [P, 2], BF16)
    bc_b = din("bc", [1, 1], FP32)
    lw0 = din("lw0", [P, 2, P], BF16)
    lw1 = din("lw1", [20, 2, P], BF16)
    lb = din("lb", [P, 2], FP32)
    leakst = din("leakst", [148, BCH], BF16)

    out = nc.declare_dram_parameter("out", [1, BCH], FP32, True)

    # internal DRAM
    x_bufs = [nc.dram_tensor(f"x{i}", [P, JX, T, BC], BF16) for i in range(2)]
    # xp buffers (bf16); bw (d=1) stored time-reversed so the scan indexes
    # both dirs identically. Layout: [P, tb, u, gate_tile, b].
    xpz = [nc.dram_tensor(f"xpz{d}", [P, TB, U, 4, BC], BF16) for d in range(2)]
    xph = [nc.dram_tensor(f"xph{d}", [P, TB, U, 2, BC], BF16) for d in range(2)]

    with tile.TileContext(nc) as tc, tc.tile_pool(name="const", bufs=1) as cpool:
        # ---- constants in SBUF
        ident = cpool.tile([P, P], BF16)
        make_identity(nc, ident[:])
        idx_sb = cpool.tile([P, NCH], I32)
        nc.sync.dma_start(idx_sb[:], idxw[:])
        pb_sb = cpool.tile([P, n_layers, 2, JG], FP32)
        nc.sync.dma_start(pb_sb[:], pbias[:])
        b1h_sb = cpool.tile([P, n_layers, 2, JH, BC], BF16)
        nc.sync.dma_start(b1h_sb[:], b1hbc[:])
        fin_hold = [cpool.tile([P, JH, 1, BC], BF16, name=f"fin{i}") for i in range(2)]

        # ---- phase 1: embedding gather -> x0 (layer-0 input, transposed)
        x0 = x_bufs[0]
        with (
            tc.tile_pool(name="erow", bufs=3) as epool,
            tc.tile_pool(name="estage", bufs=2) as espool,
            tc.tile_pool(name="epsum", bufs=4, space="PSUM") as eppool,
        ):
            for tc_i in range(T // P):
                stages = [
                    espool.tile([P, P, BC], BF16, name=f"estg{j}", tag=f"st{j}")
                    for j in range(JX)
                ]
                for bi in range(BC):
                    ch = bi * (T // P) + tc_i
                    g = epool.tile([P, E], BF16)
                    nc.gpsimd.indirect_dma_start(
                        out=g[:],
                        out_offset=None,
                        in_=emb[:],
                        in_offset=bass.IndirectOffsetOnAxis(
                            ap=idx_sb[:, ch : ch + 1], axis=0
                        ),
                    )
                    for j in range(JX):
                        pst = eppool.tile([P, P], BF16)
                        nc.tensor.transpose(pst[:], g[:, j * P : (j + 1) * P], ident[:])
                        if (bi + j) % 2 == 0:
                            nc.vector.tensor_copy(stages[j][:, :, bi], pst[:])
                        else:
                            nc.scalar.copy(stages[j][:, :, bi], pst[:])
                for j in range(JX):
                    nc.sync.dma_start(
                        x0[:, j, tc_i * P : (tc_i + 1) * P, :], stages[j][:]
                    )

        # ---- per-layer: proj (both dirs) then scan (both dirs)
        for l in range(n_layers):
            x_cur = x_bufs[l % 2]
            x_next = x_bufs[(l + 1) % 2]
            is_last = l == n_layers - 1

            # -- input projection: xp^T = Wx^T @ x^T (+bias), to DRAM (bf16)
            with (
                tc.tile_pool(name="wts", bufs=1) as wpool,
                tc.tile_pool(name="pstage", bufs=3) as pspool,
                tc.tile_pool(name="ppsum", bufs=2, space="PSUM") as pppool,
                tc.tile_pool(name="xchunk", bufs=2) as xcpool,
            ):
                wx_sb = wpool.tile([P, 2, JX, JG, P], BF16)
                nc.sync.dma_start(wx_sb[:], wx[l].rearrange("d kt mt p q -> p d kt mt q"))

                NCK = T // 64  # chunks of 512 psum cols (4 tb x 16 u x 8 b)
                NTBC = 64 // U  # tb blocks per chunk (4)
                for d in range(2):
                    for c in range(NCK):
                        xch = xcpool.tile([P, JX, 64, BC], BF16, tag="xch")
                        nc.sync.dma_start(xch[:], x_cur[:, :, c * 64 : (c + 1) * 64, :])
                        for mt in range(JG):
                            ps = pppool.tile([P, 512], FP32)
                            for kt in range(JX):
                                nc.tensor.matmul(
                                    ps[:],
                                    wx_sb[:, d, kt, mt, :],
                                    xch[:, kt, :, :],
                                    start=(kt == 0),
                                    stop=(kt == JX - 1),
                                )
                            # psum col order = (tb, u, b); stg mirrors it
                            stg = pspool.tile([P, NTBC, U, BC], BF16, tag="stg")
                            if d == 0:
                                stg_w = stg[:]
                                dst_tb = slice(c * NTBC, (c + 1) * NTBC)
                            else:
                                # bw: store reversed in time (block and
                                # within-block order both reversed)
                                stg_w = stg[:, ::-1, ::-1, :]
                                dst_tb = slice(TB - (c + 1) * NTBC, TB - c * NTBC)
                            if mt < 4:
                                nc.vector.tensor_scalar_add(
                                    stg_w, ps[:], pb_sb[:, l, d, mt : mt + 1]
                                )
                                dst = xpz[d][:, dst_tb, :, mt, :]
                            else:
                                nc.scalar.activation(
                                    stg_w,
                                    ps[:],
                                    AF.Identity,
                                    bias=pb_sb[:, l, d, mt : mt + 1],
                                )
                                dst = xph[d][:, dst_tb, :, mt - 4, :]
                            nc.sync.dma_start(dst, stg[:])

            # -- scan
            with (
                tc.tile_pool(name="state", bufs=1) as stpool,
                tc.tile_pool(name="gates", bufs=3) as gpool,
                tc.tile_pool(name="xpchunk", bufs=2) as xppool,
                tc.tile_pool(name="spsum", bufs=2, space="PSUM") as sppool,
                tc.tile_pool(name="wts2", bufs=1) as wpool2,
            ):
                wh_sb = wpool2.tile([P, 2, KH, JG, P], BF16)
                nc.sync.dma_start(wh_sb[:], wh[l].rearrange("d kt mt p q -> p d kt mt q"))

                # state ring [P, KH, U, BC]: fw writes step u to col u; bw
                # writes step u to col U-1-u so ring cols are ascending in t
                # and the x_next DMA is a plain 3D copy. Carry-in: fw reads
                # col U-1 of the previous iteration at u=0, bw reads col 0.
                # memset once: t=0 reads zeros.
                stg_st = []
                for d in range(2):
                    s = stpool.tile([P, KH, U, BC], BF16, name=f"ring{d}")
                    nc.vector.memset(s[:], 0.0)
                    stg_st.append(s)

                def rcol(d, u):
                    # ring column holding the state produced by step u
                    return u if d == 0 else U - 1 - u

                def scan_block(ib, dyn):
                    xzc, xhc = [], []
                    for d in range(2):
                        bsl = bass.ds(ib, 1) if dyn else slice(ib, ib + 1)
                        xz = xppool.tile([P, 1, U, 4, BC], BF16, tag=f"xz{d}")
                        nc.sync.dma_start(xz[:], xpz[d][:, bsl, :, :, :])
                        xh_ = xppool.tile([P, 1, U, 2, BC], BF16, tag=f"xh{d}")
                        nc.sync.dma_start(xh_[:], xph[d][:, bsl, :, :, :])
                        xzc.append(xz)
                        xhc.append(xh_)

                    def emit_xnext(d, half):
                        # src: ring cols [half*8, half*8+8), already ascending
                        # in t for both dirs -> plain 3D copy
                        lo = half * 8
                        src = stg_st[d][:, :, lo : lo + 8, :]
                        if d == 0:
                            t0 = ib * U + lo
                            t_ap = bass.ds(t0, 8) if dyn else slice(ib * U + lo, ib * U + lo + 8)
                        else:
                            # ring col c holds t = T - U - ib*U + c
                            base = T - U + lo
                            t_ap = (
                                bass.ds(ib * (-U) + base, 8)
                                if dyn
                                else slice(base - ib * U, base - ib * U + 8)
                            )
                        nc.sync.dma_start(x_next[:, 2 * d : 2 * d + 2, t_ap, :], src)

                    for u in range(U):
                        pts = []
                        for d in range(2):
                            up = rcol(d, u - 1) if u > 0 else rcol(d, U - 1)
                            pt = sppool.tile([P, JG, BC], FP32, tag=f"ps{d}")
                            nc.tensor.matmul(
                                pt[:, 0:4, :], ident[:], xzc[d][:, 0, u, :, :],
                                start=True, stop=False, skip_group_check=True,
                            )
                            nc.tensor.matmul(
                                pt[:, 4:6, :], ident[:], b1h_sb[:, l, d, :, :],
                                start=True, stop=False, skip_group_check=True,
                            )
                            for mt in range(JG):
                                for kt in range(KH):
                                    nc.tensor.matmul(
                                        pt[:, mt, :],
                                        wh_sb[:, d, kt, mt, :],
                                        stg_st[d][:, kt, up, :],
                                        start=False,
                                        stop=(kt == KH - 1),
                                        skip_group_check=True,
                                    )
                            pts.append(pt)
                        zrs, hms, avs, hhs, dds, ees = [], [], [], [], [], []
                        for d in range(2):
                            zr = gpool.tile([P, 4, BC], FP32, tag=f"zr{d}")
                            nc.scalar.activation(zr[:], pts[d][:, 0:4, :], AF.Sigmoid)
                            zrs.append(zr)
                        for d in range(2):
                            hm = gpool.tile([P, JH, BC], FP32, tag=f"hm{d}")
                            nc.vector.tensor_tensor(
                                hm[:], pts[d][:, 4:6, :], zrs[d][:, 2:4, :], OP.mult
                            )
                            hms.append(hm)
                        for d in range(2):
                            av = gpool.tile([P, JH, BC], FP32, tag=f"av{d}")
                            nc.gpsimd.tensor_tensor(
                                av[:], hms[d][:], xhc[d][:, 0, u, :, :], OP.add
                            )
                            avs.append(av)
                        for d in range(2):
                            hh = gpool.tile([P, JH, BC], FP32, tag=f"hh{d}")
                            nc.scalar.activation(hh[:], avs[d][:], AF.Tanh)
                            hhs.append(hh)
                        for d in range(2):
                            up = rcol(d, u - 1) if u > 0 else rcol(d, U - 1)
                            dd = gpool.tile([P, JH, BC], FP32, tag=f"dd{d}")
                            nc.gpsimd.tensor_tensor(
                                dd[:], stg_st[d][:, :, up, :], hhs[d][:], OP.subtract
                            )
                            dds.append(dd)
                        for d in range(2):
                            ee = gpool.tile([P, JH, BC], FP32, tag=f"ee{d}")
                            nc.vector.tensor_tensor(
                                ee[:], zrs[d][:, 0:2, :], dds[d][:], OP.mult
                            )
                            ees.append(ee)
                        for d in range(2):
                            nc.vector.tensor_tensor(
                                stg_st[d][:, :, rcol(d, u), :],
                                hhs[d][:],
                                ees[d][:],
                                OP.add,
                            )
                        if not is_last and u == U // 2 - 1:
                            # fw has filled cols 0:8, bw cols 8:16
                            emit_xnext(0, 0)
                            emit_xnext(1, 1)
                    if not is_last:
                        emit_xnext(0, 1)
                        emit_xnext(1, 0)

                if use_for_i:
                    with tc.For_i(0, TB, 1, staggered_reset=staggered) as ib:
                        scan_block(ib, True)
                else:
                    for ib in range(TB):
                        scan_block(ib, False)

                if is_last:
                    for d in range(2):
                        nc.vector.tensor_copy(
                            fin_hold[d][:, :, 0, :],
                            stg_st[d][:, :, rcol(d, U - 1), :],
                        )

        # ---- head: leaks branch + folded BN/FC/BN/sigmoid
        with (
            tc.tile_pool(name="head", bufs=1) as hpool,
            tc.tile_pool(name="hpsum", bufs=2, space="PSUM") as hppool,
        ):
            lkw0 = hpool.tile([P, 2, P], BF16)
            nc.sync.dma_start(lkw0[:], lw0[:])
            lkw1 = hpool.tile([20, 2, P], BF16)
            nc.sync.dma_start(lkw1[:], lw1[:])
            lkb = hpool.tile([P, 2], FP32)
            nc.sync.dma_start(lkb[:], lb[:])
            lkx0 = hpool.tile([P, BCH], BF16)
            nc.sync.dma_start(lkx0[:], leakst[0:P, :])
            lkx1 = hpool.tile([20, BCH], BF16)
            nc.sync.dma_start(lkx1[:], leakst[P:148, :])

            lks = hpool.tile([P, 2, BCH], BF16)
            for mt in range(2):
                lp = hppool.tile([P, BCH], FP32, tag="lp")
                nc.tensor.matmul(lp[:], lkw0[:, mt, :], lkx0[:], start=True, stop=False)
                nc.tensor.matmul(lp[:], lkw1[:, mt, :], lkx1[:], start=False, stop=True)
                nc.scalar.activation(
                    lks[:, mt, :], lp[:], AF.Relu, bias=lkb[:, mt : mt + 1]
                )

            w1_sb = hpool.tile([P, 10, 2, P], BF16)
            nc.sync.dma_start(w1_sb[:], w1[:].rearrange("kt mt p q -> p kt mt q"))
            b1_sb = hpool.tile([P, 2], FP32)
            nc.sync.dma_start(b1_sb[:], b1p[:])
            wc_sb = hpool.tile([P, 2], BF16)
            nc.sync.dma_start(wc_sb[:], wc[:])
            bc_sb = hpool.tile([1, 1], FP32)
            nc.sync.dma_start(bc_sb[:], bc_b[:])

            sf, sb_ = fin_hold
            rhs_tiles = []
            for half in range(2):  # code (cols 0:4), comment (cols 4:8)
                c0 = half * BCH
                for dstate in (sf, sb_):
                    for j in range(JH):
                        rhs_tiles.append(dstate[:, j, 0, c0 : c0 + BCH])
            rhs_tiles.append(lks[:, 0, :])
            rhs_tiles.append(lks[:, 1, :])

            yt = hpool.tile([P, 2, BCH], BF16)
            for mt in range(2):
                hp = hppool.tile([P, BCH], FP32, tag="hp")
                for kt in range(10):
                    nc.tensor.matmul(
                        hp[:],
                        w1_sb[:, kt, mt, :],
                        rhs_tiles[kt],
                        start=(kt == 0),
                        stop=(kt == 9),
                    )
                nc.scalar.activation(
                    yt[:, mt, :], hp[:], AF.Relu, bias=b1_sb[:, mt : mt + 1]
                )

            op_ = hppool.tile([1, BCH], FP32, tag="op")
            for kt in range(2):
                nc.tensor.matmul(
                    op_[:],
                    wc_sb[:, kt : kt + 1],
                    yt[:, kt, :],
                    start=(kt == 0),
                    stop=(kt == 1),
                )
            res = hpool.tile([1, BCH], FP32)
            nc.scalar.activation(res[:], op_[:], AF.Sigmoid, bias=bc_sb[0:1, 0:1])
            nc.sync.dma_start(out[:], res[:])

    nc.compile()
    return nc


def prep_inputs(inputs, T=512, n_layers=NLAY):
    """Host-side: shard + pre-layout all tensors. Returns in_maps list."""
    ci = np.asarray(inputs["comment_indices"]).astype(np.int32)
    co = np.asarray(inputs["code_indices"]).astype(np.int32)
    emb_bf = np.ascontiguousarray(
        np.asarray(inputs["embed_table"], np.float32)
    ).astype(NP_BF16)
    gwx = np.asarray(inputs["gru_Wx"], np.float32)
    gwh = np.asarray(inputs["gru_Wh"], np.float32)
    gb = np.asarray(inputs["gru_b"], np.float32)

    wx_t = np.ascontiguousarray(
        gwx[:n_layers].reshape(n_layers, 2, JX, P, JG, P).transpose(0, 1, 2, 4, 3, 5)
    ).astype(NP_BF16)
    wh_t = np.ascontiguousarray(
        gwh[:n_layers].reshape(n_layers, 2, KH, P, JG, P).transpose(0, 1, 2, 4, 3, 5)
    ).astype(NP_BF16)

    pb = gb[:n_layers, :, 0, :].copy()  # [nl, 2, 768]
    pb[:, :, : 2 * H] += gb[:n_layers, :, 1, : 2 * H]
    pbias_h = np.ascontiguousarray(
        pb.reshape(n_layers, 2, JG, P).transpose(3, 0, 1, 2)
    ).astype(np.float32)
    b1h = np.ascontiguousarray(
        gb[:n_layers, :, 1, 2 * H :].reshape(n_layers, 2, JH, P).transpose(3, 0, 1, 2)
    ).astype(np.float32)  # [P, nl, 2, JH]
    b1hbc_h = np.ascontiguousarray(
        np.broadcast_to(b1h[..., None], (P, n_layers, 2, JH, BC))
    ).astype(NP_BF16)

    s1 = np.asarray(inputs["bn1_gamma"], np.float32) / np.sqrt(
        np.asarray(inputs["bn1_var"], np.float32) + EPS
    )
    t1 = (
        np.asarray(inputs["bn1_beta"], np.float32)
        - np.asarray(inputs["bn1_mean"], np.float32) * s1
    )
    fc1 = np.asarray(inputs["fc1_W"], np.float32)
    w1p = fc1 * s1[:, None]
    b1v = t1 @ fc1 + np.asarray(inputs["fc1_b"], np.float32)
    s2 = np.asarray(inputs["bn2_gamma"], np.float32) / np.sqrt(
        np.asarray(inputs["bn2_var"], np.float32) + EPS
    )
    t2 = (
        np.asarray(inputs["bn2_beta"], np.float32)
        - np.asarray(inputs["bn2_mean"], np.float32) * s2
    )
    clsw = np.asarray(inputs["cls_W"], np.float32)
    wcp = clsw * s2[:, None]
    bcp = (t2 @ clsw + np.asarray(inputs["cls_b"], np.float32)).reshape(1, 1)

    w1_t = np.ascontiguousarray(w1p.reshape(10, P, 2, P).transpose(0, 2, 1, 3)).astype(
        NP_BF16
    )
    b1p_h = np.ascontiguousarray(b1v.reshape(2, P).T).astype(np.float32)
    wc_h = np.ascontiguousarray(wcp.reshape(2, P).T).astype(NP_BF16)

    lw = np.asarray(inputs["leaks_W"], np.float32)
    lw0_h = np.ascontiguousarray(lw[:P].reshape(P, 2, P)).astype(NP_BF16)
    lw1_h = np.ascontiguousarray(lw[P:].reshape(20, 2, P)).astype(NP_BF16)
    lb_h = np.ascontiguousarray(
        np.asarray(inputs["leaks_b"], np.float32).reshape(2, P).T
    ).astype(np.float32)
    leaks = np.asarray(inputs["leaks_indices"], np.float32)

    shared = dict(
        emb=emb_bf, wx=wx_t, wh=wh_t, pbias=pbias_h, b1hbc=b1hbc_h,
        w1=w1_t, b1p=b1p_h, wc=wc_h, bc=bcp.astype(np.float32),
        lw0=lw0_h, lw1=lw1_h, lb=lb_h,
    )
    in_maps = []
    for c in range(NCORES):
        exs = slice(BCH * c, BCH * c + BCH)
        merged = np.concatenate([co[exs, :T], ci[exs, :T]], 0)  # [8, T]
        idxw_h = np.ascontiguousarray(
            merged.reshape(BC, T // P, P).transpose(2, 0, 1).reshape(P, -1)
        ).astype(np.int32)
        lkt = np.ascontiguousarray(leaks[exs].T).astype(NP_BF16)
        m = dict(shared)
        m["idxw"] = idxw_h
        m["leakst"] = lkt
        in_maps.append(m)
    return in_maps


def kernel(**inputs) -> np.ndarray:
    from concourse.bass_utils import run_bass_kernel_spmd

    nc = build_nc(T=512)
    in_maps = prep_inputs(inputs, T=512)
    res = run_bass_kernel_spmd(nc, in_maps, list(range(NCORES)))
    outs = [np.asarray(res.results[c]["out"]).reshape(-1) for c in range(NCORES)]
    return np.concatenate(outs).astype(np.float32)


if __name__ == "__main__":
    sys.path.insert(0, "/root/problem")
    import reference

    inp = {k: np.asarray(v) for k, v in reference.setup_inputs().items()}
    got = kernel(**inp)
    print("kernel out:", got[:8])


# revision 11
# speedup vs baseline: 2.6923x; 2.6923x over previous
"""Trainium2 Bass kernel for nn_BaselineModel_27298812133937.

Model: two [32,512] token sequences -> shared embedding [50000,512] ->
3 stacked bi-GRU layers (H=256, Keras reset_after) -> last states,
plus a leaks MLP branch, then BN/FC/BN/sigmoid head -> [32].

Sharding: the two sequences share GRU weights, so they merge into a
batch of 64. Each of the 8 cores takes 8 merged examples (4 code + 4
comment of the same original examples), runs the full network for its
shard with no cross-core communication, and computes the head for its
4 original examples. Host concatenates the 8x[4] outputs.

On-core layout: feature dim on partitions, batch on the free dim. The
recurrent matmul keeps Wh (bf16) stationary and streams the state.
v2 scan: xz and the recurrent h-bias are accumulated into PSUM by
identity-stationary matmuls (PE is cheap; the gate chain is the
bottleneck), the per-direction gate chain is 2 Act + 3 DVE + 2 GpSimd
ops, and the hidden state lives in a [P, KH, BC, U] SBUF ring staged
to DRAM in half-block DMAs instead of per-step writes.
"""

import os
import sys

import numpy as np

for _p in ("/opt/trn_rl_repo",):
    if os.path.isdir(_p) and _p not in sys.path:
        sys.path.insert(0, _p)

import concourse.bass as bass
import concourse.tile as tile
from concourse import bacc, mybir
from concourse.masks import make_identity

import ml_dtypes

FP32 = mybir.dt.float32
BF16 = mybir.dt.bfloat16
I32 = mybir.dt.int32
AF = mybir.ActivationFunctionType
OP = mybir.AluOpType
NP_BF16 = ml_dtypes.bfloat16

V, E, H, NLAY = 50000, 512, 256, 3
EPS = 1e-3
P = 128
JX = E // P        # 4  x-feature tiles
JG = 3 * H // P    # 6  gate tiles
JH = H // P        # 2  hidden tiles
KH = H // P        # 2  Wh contraction tiles
BC = 8             # merged examples per core
BCH = 4            # head (original) examples per core
NCORES = 8
U = 16             # scan steps per For_i iteration == xp time-block


def build_nc(T=512, n_layers=NLAY, use_for_i=True, staggered=True, debug=False):
    assert T % P == 0 and T % U == 0
    TB = T // U
    NCH = BC * (T // P)

    nc = bacc.Bacc("TRN2", target_bir_lowering=False, debug=debug)

    def din(name, shape, dt):
        return nc.declare_dram_parameter(name, list(shape), dt, False)

    emb = din("emb", [V, E], BF16)
    idxw = din("idxw", [P, NCH], I32)
    wx = din("wx", [n_layers, 2, JX, JG, P, P], BF16)
    wh = din("wh", [n_layers, 2, KH, JG, P, P], BF16)
    pbias = din("pbias", [P, n_layers, 2, JG], FP32)
    b1hbc = din("b1hbc", [P, n_layers, 2, JH, BC], BF16)
    w1 = din("w1", [10, 2, P, P], BF16)
    b1p = din("b1p", [P, 2], FP32)
    wc = din("wc", [P, 2], BF16)
    bc_b = din("bc", [1, 1], FP32)
    lw0 = din("lw0", [P, 2, P], BF16)
    lw1 = din("lw1", [20, 2, P], BF16)
    lb = din("lb", [P, 2], FP32)
    leakst = din("leakst", [148, BCH], BF16)

    out = nc.declare_dram_parameter("out", [1, BCH], FP32, True)


    with tile.TileContext(nc) as tc, tc.tile_pool(name="const", bufs=1) as cpool:
        # ---- constants in SBUF
        ident = cpool.tile([P, P], BF16)
        make_identity(nc, ident[:])
        idx_sb = cpool.tile([P, NCH], I32)
        nc.sync.dma_start(idx_sb[:], idxw[:])
        pb_sb = cpool.tile([P, n_layers, 2, JG], FP32)
        nc.sync.dma_start(pb_sb[:], pbias[:])
        b1h_sb = cpool.tile([P, n_layers, 2, JH, BC], BF16)
        nc.sync.dma_start(b1h_sb[:], b1hbc[:])
        fin_hold = [cpool.tile([P, JH, 1, BC], BF16, name=f"fin{i}") for i in range(2)]
        # layer activations and input projections live entirely in SBUF:
        # x_sb [P, 4, T, BC] (single buffer: proj l reads it fully before
        # scan l overwrites it); xp_sb [P, dir, gate_tile, TB, U, BC] with
        # bw stored time-reversed so the scan indexes both dirs identically.
        x_sb = cpool.tile([P, JX, T, BC], BF16, name="x_sb")
        xp_sb = cpool.tile([P, 2, JG, TB, U, BC], BF16, name="xp_sb")

        # ---- phase 1: embedding gather -> x_sb (layer-0 input, transposed)
        with (
            tc.tile_pool(name="erow", bufs=3) as epool,
            tc.tile_pool(name="epsum", bufs=4, space="PSUM") as eppool,
        ):
            for tc_i in range(T // P):
                for bi in range(BC):
                    ch = bi * (T // P) + tc_i
                    g = epool.tile([P, E], BF16)
                    nc.gpsimd.indirect_dma_start(
                        out=g[:],
                        out_offset=None,
                        in_=emb[:],
                        in_offset=bass.IndirectOffsetOnAxis(
                            ap=idx_sb[:, ch : ch + 1], axis=0
                        ),
                    )
                    for j in range(JX):
                        pst = eppool.tile([P, P], BF16)
                        nc.tensor.transpose(pst[:], g[:, j * P : (j + 1) * P], ident[:])
                        dst = x_sb[:, j, tc_i * P : (tc_i + 1) * P, bi]
                        if (bi + j) % 2 == 0:
                            nc.vector.tensor_copy(dst, pst[:])
                        else:
                            nc.scalar.copy(dst, pst[:])

        # ---- per-layer: proj (both dirs) then scan (both dirs)
        for l in range(n_layers):
            is_last = l == n_layers - 1

            # -- input projection: xp^T = Wx^T @ x^T (+bias), psum -> xp_sb
            with (
                tc.tile_pool(name="wts", bufs=1) as wpool,
                tc.tile_pool(name="ppsum", bufs=2, space="PSUM") as pppool,
            ):
                wx_sb = wpool.tile([P, 2, JX, JG, P], BF16)
                nc.sync.dma_start(wx_sb[:], wx[l].rearrange("d kt mt p q -> p d kt mt q"))

                NCK = T // 64  # chunks of 512 psum cols (4 tb x 16 u x 8 b)
                NTBC = 64 // U  # tb blocks per chunk (4)
                for d in range(2):
                    for c in range(NCK):
                        for mt in range(JG):
                            ps = pppool.tile([P, 512], FP32)
                            for kt in range(JX):
                                nc.tensor.matmul(
                                    ps[:],
                                    wx_sb[:, d, kt, mt, :],
                                    x_sb[:, kt, c * 64 : (c + 1) * 64, :],
                                    start=(kt == 0),
                                    stop=(kt == JX - 1),
                                )
                            # psum col order = (tb, u, b) -> xp_sb slice
                            if d == 0:
                                dst = xp_sb[:, 0, mt, c * NTBC : (c + 1) * NTBC, :, :]
                            else:
                                # bw: store reversed in time (block and
                                # within-block order both reversed)
                                dst = xp_sb[
                                    :, 1, mt, TB - (c + 1) * NTBC : TB - c * NTBC, :, :
                                ][:, ::-1, ::-1, :]
                            if mt < 4:
                                nc.vector.tensor_scalar_add(
                                    dst, ps[:], pb_sb[:, l, d, mt : mt + 1]
                                )
                            else:
                                nc.scalar.activation(
                                    dst,
                                    ps[:],
                                    AF.Identity,
                                    bias=pb_sb[:, l, d, mt : mt + 1],
                                )

            # -- scan
            with (
                tc.tile_pool(name="state", bufs=1) as stpool,
                tc.tile_pool(name="gates", bufs=3) as gpool,
                tc.tile_pool(name="spsum", bufs=2, space="PSUM") as sppool,
                tc.tile_pool(name="wts2", bufs=1) as wpool2,
            ):
                wh_sb = wpool2.tile([P, 2, KH, JG, P], BF16)
                nc.sync.dma_start(wh_sb[:], wh[l].rearrange("d kt mt p q -> p d kt mt q"))

                # state ring [P, KH, U, BC]: fw writes step u to col u; bw
                # writes step u to col U-1-u so ring cols are ascending in t
                # and the x_next DMA is a plain 3D copy. Carry-in: fw reads
                # col U-1 of the previous iteration at u=0, bw reads col 0.
                # memset once: t=0 reads zeros.
                stg_st = []
                for d in range(2):
                    s = stpool.tile([P, KH, U, BC], BF16, name=f"ring{d}")
                    nc.vector.memset(s[:], 0.0)
                    stg_st.append(s)

                def rcol(d, u):
                    # ring column holding the state produced by step u
                    return u if d == 0 else U - 1 - u

                def scan_block(ib):
                    def emit_xnext(d, half):
                        # ring cols [half*8, half*8+8), ascending in t for
                        # both dirs -> on-chip copy into x_sb
                        lo = half * 8
                        src = stg_st[d][:, :, lo : lo + 8, :]
                        if d == 0:
                            t0 = ib * U + lo
                        else:
                            t0 = T - U + lo - ib * U
                        dst = x_sb[:, 2 * d : 2 * d + 2, t0 : t0 + 8, :]
                        nc.gpsimd.tensor_copy(dst, src)

                    for u in range(U):
                        pts = []
                        for d in range(2):
                            up = rcol(d, u - 1) if u > 0 else rcol(d, U - 1)
                            pt = sppool.tile([P, JG, BC], FP32, tag=f"ps{d}")
                            nc.tensor.matmul(
                                pt[:, 0:4, :], ident[:],
                                xp_sb[:, d, 0:4, ib, u, :],
                                start=True, stop=False, skip_group_check=True,
                            )
                            nc.tensor.matmul(
                                pt[:, 4:6, :], ident[:], b1h_sb[:, l, d, :, :],
                                start=True, stop=False, skip_group_check=True,
                            )
                            for mt in range(JG):
                                for kt in range(KH):
                                    nc.tensor.matmul(
                                        pt[:, mt, :],
                                        wh_sb[:, d, kt, mt, :],
                                        stg_st[d][:, kt, up, :],
                                        start=False,
                                        stop=(kt == KH - 1),
                                        skip_group_check=True,
                                    )
                            pts.append(pt)
                        zrs, hms, avs, hhs, dds, ees = [], [], [], [], [], []
                        for d in range(2):
                            zr = gpool.tile([P, 4, BC], FP32, tag=f"zr{d}")
                            nc.scalar.activation(zr[:], pts[d][:, 0:4, :], AF.Sigmoid)
                            zrs.append(zr)
                        for d in range(2):
                            hm = gpool.tile([P, JH, BC], FP32, tag=f"hm{d}")
                            nc.vector.tensor_tensor(
                                hm[:], pts[d][:, 4:6, :], zrs[d][:, 2:4, :], OP.mult
                            )
                            hms.append(hm)
                        for d in range(2):
                            av = gpool.tile([P, JH, BC], FP32, tag=f"av{d}")
                            nc.gpsimd.tensor_tensor(
                                av[:], hms[d][:], xp_sb[:, d, 4:6, ib, u, :], OP.add
                            )
                            avs.append(av)
                        for d in range(2):
                            hh = gpool.tile([P, JH, BC], FP32, tag=f"hh{d}")
                            nc.scalar.activation(hh[:], avs[d][:], AF.Tanh)
                            hhs.append(hh)
                        for d in range(2):
                            up = rcol(d, u - 1) if u > 0 else rcol(d, U - 1)
                            dd = gpool.tile([P, JH, BC], FP32, tag=f"dd{d}")
                            nc.gpsimd.tensor_tensor(
                                dd[:], stg_st[d][:, :, up, :], hhs[d][:], OP.subtract
                            )
                            dds.append(dd)
                        for d in range(2):
                            ee = gpool.tile([P, JH, BC], FP32, tag=f"ee{d}")
                            nc.vector.tensor_tensor(
                                ee[:], zrs[d][:, 0:2, :], dds[d][:], OP.mult
                            )
                            ees.append(ee)
                        for d in range(2):
                            nc.vector.tensor_tensor(
                                stg_st[d][:, :, rcol(d, u), :],
                                hhs[d][:],
                                ees[d][:],
                                OP.add,
                            )
                        if not is_last and u == U // 2 - 1:
                            # fw has filled cols 0:8, bw cols 8:16
                            emit_xnext(0, 0)
                            emit_xnext(1, 1)
                    if not is_last:
                        emit_xnext(0, 1)
                        emit_xnext(1, 0)

                for ib in range(TB):
                    scan_block(ib)

                if is_last:
                    for d in range(2):
                        nc.vector.tensor_copy(
                            fin_hold[d][:, :, 0, :],
                            stg_st[d][:, :, rcol(d, U - 1), :],
                        )

        # ---- head: leaks branch + folded BN/FC/BN/sigmoid
        with (
            tc.tile_pool(name="head", bufs=1) as hpool,
            tc.tile_pool(name="hpsum", bufs=2, space="PSUM") as hppool,
        ):
            lkw0 = hpool.tile([P, 2, P], BF16)
            nc.sync.dma_start(lkw0[:], lw0[:])
            lkw1 = hpool.tile([20, 2, P], BF16)
            nc.sync.dma_start(lkw1[:], lw1[:])
            lkb = hpool.tile([P, 2], FP32)
            nc.sync.dma_start(lkb[:], lb[:])
            lkx0 = hpool.tile([P, BCH], BF16)
            nc.sync.dma_start(lkx0[:], leakst[0:P, :])
            lkx1 = hpool.tile([20, BCH], BF16)
            nc.sync.dma_start(lkx1[:], leakst[P:148, :])

            lks = hpool.tile([P, 2, BCH], BF16)
            for mt in range(2):
                lp = hppool.tile([P, BCH], FP32, tag="lp")
                nc.tensor.matmul(lp[:], lkw0[:, mt, :], lkx0[:], start=True, stop=False)
                nc.tensor.matmul(lp[:], lkw1[:, mt, :], lkx1[:], start=False, stop=True)
                nc.scalar.activation(
                    lks[:, mt, :], lp[:], AF.Relu, bias=lkb[:, mt : mt + 1]
                )

            w1_sb = hpool.tile([P, 10, 2, P], BF16)
            nc.sync.dma_start(w1_sb[:], w1[:].rearrange("kt mt p q -> p kt mt q"))
            b1_sb = hpool.tile([P, 2], FP32)
            nc.sync.dma_start(b1_sb[:], b1p[:])
            wc_sb = hpool.tile([P, 2], BF16)
            nc.sync.dma_start(wc_sb[:], wc[:])
            bc_sb = hpool.tile([1, 1], FP32)
            nc.sync.dma_start(bc_sb[:], bc_b[:])

            sf, sb_ = fin_hold
            rhs_tiles = []
            for half in range(2):  # code (cols 0:4), comment (cols 4:8)
                c0 = half * BCH
                for dstate in (sf, sb_):
                    for j in range(JH):
                        rhs_tiles.append(dstate[:, j, 0, c0 : c0 + BCH])
            rhs_tiles.append(lks[:, 0, :])
            rhs_tiles.append(lks[:, 1, :])

            yt = hpool.tile([P, 2, BCH], BF16)
            for mt in range(2):
                hp = hppool.tile([P, BCH], FP32, tag="hp")
                for kt in range(10):
                    nc.tensor.matmul(
                        hp[:],
                        w1_sb[:, kt, mt, :],
                        rhs_tiles[kt],
                        start=(kt == 0),
                        stop=(kt == 9),
                    )
                nc.scalar.activation(
                    yt[:, mt, :], hp[:], AF.Relu, bias=b1_sb[:, mt : mt + 1]
                )

            op_ = hppool.tile([1, BCH], FP32, tag="op")
            for kt in range(2):
                nc.tensor.matmul(
                    op_[:],
                    wc_sb[:, kt : kt + 1],
                    yt[:, kt, :],
                    start=(kt == 0),
                    stop=(kt == 1),
                )
            res = hpool.tile([1, BCH], FP32)
            nc.scalar.activation(res[:], op_[:], AF.Sigmoid, bias=bc_sb[0:1, 0:1])
            nc.sync.dma_start(out[:], res[:])

    nc.compile()
    return nc


def prep_inputs(inputs, T=512, n_layers=NLAY):
    """Host-side: shard + pre-layout all tensors. Returns in_maps list."""
    ci = np.asarray(inputs["comment_indices"]).astype(np.int32)
    co = np.asarray(inputs["code_indices"]).astype(np.int32)
    emb_bf = np.ascontiguousarray(
        np.asarray(inputs["embed_table"], np.float32)
    ).astype(NP_BF16)
    gwx = np.asarray(inputs["gru_Wx"], np.float32)
    gwh = np.asarray(inputs["gru_Wh"], np.float32)
    gb = np.asarray(inputs["gru_b"], np.float32)

    wx_t = np.ascontiguousarray(
        gwx[:n_layers].reshape(n_layers, 2, JX, P, JG, P).transpose(0, 1, 2, 4, 3, 5)
    ).astype(NP_BF16)
    wh_t = np.ascontiguousarray(
        gwh[:n_layers].reshape(n_layers, 2, KH, P, JG, P).transpose(0, 1, 2, 4, 3, 5)
    ).astype(NP_BF16)

    pb = gb[:n_layers, :, 0, :].copy()  # [nl, 2, 768]
    pb[:, :, : 2 * H] += gb[:n_layers, :, 1, : 2 * H]
    pbias_h = np.ascontiguousarray(
        pb.reshape(n_layers, 2, JG, P).transpose(3, 0, 1, 2)
    ).astype(np.float32)
    b1h = np.ascontiguousarray(
        gb[:n_layers, :, 1, 2 * H :].reshape(n_layers, 2, JH, P).transpose(3, 0, 1, 2)
    ).astype(np.float32)  # [P, nl, 2, JH]
    b1hbc_h = np.ascontiguousarray(
        np.broadcast_to(b1h[..., None], (P, n_layers, 2, JH, BC))
    ).astype(NP_BF16)

    s1 = np.asarray(inputs["bn1_gamma"], np.float32) / np.sqrt(
        np.asarray(inputs["bn1_var"], np.float32) + EPS
    )
    t1 = (
        np.asarray(inputs["bn1_beta"], np.float32)
        - np.asarray(inputs["bn1_mean"], np.float32) * s1
    )
    fc1 = np.asarray(inputs["fc1_W"], np.float32)
    w1p = fc1 * s1[:, None]
    b1v = t1 @ fc1 + np.asarray(inputs["fc1_b"], np.float32)
    s2 = np.asarray(inputs["bn2_gamma"], np.float32) / np.sqrt(
        np.asarray(inputs["bn2_var"], np.float32) + EPS
    )
    t2 = (
        np.asarray(inputs["bn2_beta"], np.float32)
        - np.asarray(inputs["bn2_mean"], np.float32) * s2
    )
    clsw = np.asarray(inputs["cls_W"], np.float32)
    wcp = clsw * s2[:, None]
    bcp = (t2 @ clsw + np.asarray(inputs["cls_b"], np.float32)).reshape(1, 1)

    w1_t = np.ascontiguousarray(w1p.reshape(10, P, 2, P).transpose(0, 2, 1, 3)).astype(
        NP_BF16
    )
    b1p_h = np.ascontiguousarray(b1v.reshape(2, P).T).astype(np.float32)
    wc_h = np.ascontiguousarray(wcp.reshape(2, P).T).astype(NP_BF16)

    lw = np.asarray(inputs["leaks_W"], np.float32)
    lw0_h = np.ascontiguousarray(lw[:P].reshape(P, 2, P)).astype(NP_BF16)
    lw1_h = np.ascontiguousarray(lw[P:].reshape(20, 2, P)).astype(NP_BF16)
    lb_h = np.ascontiguousarray(
        np.asarray(inputs["leaks_b"], np.float32).reshape(2, P).T
    ).astype(np.float32)
    leaks = np.asarray(inputs["leaks_indices"], np.float32)

    shared = dict(
        emb=emb_bf, wx=wx_t, wh=wh_t, pbias=pbias_h, b1hbc=b1hbc_h,
        w1=w1_t, b1p=b1p_h, wc=wc_h, bc=bcp.astype(np.float32),
        lw0=lw0_h, lw1=lw1_h, lb=lb_h,
    )
    in_maps = []
    for c in range(NCORES):
        exs = slice(BCH * c, BCH * c + BCH)
        merged = np.concatenate([co[exs, :T], ci[exs, :T]], 0)  # [8, T]
        idxw_h = np.ascontiguousarray(
            merged.reshape(BC, T // P, P).transpose(2, 0, 1).reshape(P, -1)
        ).astype(np.int32)
        lkt = np.ascontiguousarray(leaks[exs].T).astype(NP_BF16)
        m = dict(shared)
        m["idxw"] = idxw_h
        m["leakst"] = lkt
        in_maps.append(m)
    return in_maps


def kernel(**inputs) -> np.ndarray:
    from concourse.bass_utils import run_bass_kernel_spmd

    nc = build_nc(T=512)
    in_maps = prep_inputs(inputs, T=512)
    res = run_bass_kernel_spmd(nc, in_maps, list(range(NCORES)))
    outs = [np.asarray(res.results[c]["out"]).reshape(-1) for c in range(NCORES)]
    return np.concatenate(outs).astype(np.float32)


if __name__ == "__main__":
    sys.path.insert(0, "/root/problem")
    import reference

    inp = {k: np.asarray(v) for k, v in reference.setup_inputs().items()}
    got = kernel(**inp)
    print("kernel out:", got[:8])
